# revision 68
# baseline (speedup 1.0000x reference)
"""Trainium2 Bass kernel for nn_GAT_38989713113447 (3-layer dense GAT).

Sharding: 8 heads over 8 cores for the two inner GAT layers (pure head
parallelism).  Out layer: Who = sum_k h1_k @ Wo[k] via a split AllReduce
([128,1584] bf16 payload that also carries the out-layer f1/f2 partial
rows); the per-core i-slice of f1o is extracted with a one-hot select
matmul (per-core `sel` input) instead of an AllToAll.  Out attention is
row-sharded (384 rows/core) with the softmax denominator folded into the
aggregation matmul as a 65th stationary column.

Math: exp(leakyrelu(s)) = max(exp(s), exp(alpha*s)) for alpha in (0,1),
and exp(f1_i + f2_j) = P_i * Q_j is rank-1.  Per attention tile the
elementwise chain is 3 ops: e2 = p_i*q_j (DVE tensor_scalar 4x-mode or
ACT Exp with per-partition bias), u = max(P_i*Q_j, e2) via ONE fused
scalar_tensor_tensor, at = u * mask via scalar_tensor_tensor (GPSIMD
gets TensorScalarPtr efficiency 0.6 vs 0.42 for plain tensor_tensor).
Work is split across ACT/DVE/GPSIMD by jb index; PE does aggregation +
ones-rowsum matmuls (attention lives transposed [j, i], contracting j).
"""

import sys

sys.path.insert(0, "/opt/trn_rl_repo")

from contextlib import ExitStack

import numpy as np
import ml_dtypes

import concourse.bass as bass  # noqa: F401
import concourse.bacc as bacc
import concourse.tile as tile
from concourse import mybir
from concourse.bass_utils import run_bass_kernel_spmd

N = 3072
F = 256
H = 8
D = 128          # H1 == H2
OUT = 64
ALPHA = 0.2
NCORES = 8
NJB = N // 128   # 24 attention j-blocks
HALF = N // 2    # i-dim half per PSUM residency
ISL = N // NCORES  # 384 output rows per core
MCH = 6          # jb per mask-chunk DMA

FP32 = mybir.dt.float32
BF16 = mybir.dt.bfloat16
AF = mybir.ActivationFunctionType
ALU = mybir.AluOpType

# engine assignment per jb: u = (g*q_j) max Q_j on DVE (4x two-op
# tensor_scalar) or GPSIMD; at = u * mask on DVE tt or GPSIMD stt
U_GP = [False for jb in range(NJB)]
AT_GP = [jb % 4 == 0 for jb in range(NJB)]   # 6 GPSIMD / 18 DVE

# AllReduce payloads (two collectives, one per layer-1 half), bf16:
#   A [128, 792]: cols 0:24 fo (jb<12, f1o/f2o interleaved), 24:792 who
#     half 0 ([64,1536] packed as [128,768], row = 2d + i//768)
#   B [128, 795]: cols 0:24 fo (jb>=12), 24:792 who half 1,
#     792:795 f1o own-slice partial ([1,384] as [128,3], i = q*128+p)
AW_A, AW_B = 792, 795
FO_OFF, WHO_OFF, F1SL_OFF = 0, 24, 792


def _ap(base, offset, dims):
    return bass.AP(base.tensor, base.offset + offset, [list(d) for d in dims])


class Builder:
    def __init__(self, nc, tc, ctx):
        self.nc = nc
        self.tc = tc
        p = lambda name, bufs, space=None: ctx.enter_context(
            tc.tile_pool(name=name, bufs=bufs, **({"space": space} if space else {}))
        )
        self.state = p("state", 1)
        self.bc = p("bc", 2)        # P_bc/p_bc (double-buffered across layers)
        self.mask = p("mask", 2)    # mask chunks
        self.work = p("work", 4)    # u tiles
        self.att = p("att", 6)
        self.misc = p("misc", 1)
        self.ps_agg = p("ps_agg", 1, "PSUM")
        self.ps_rs = p("ps_rs", 1, "PSUM")
        self.ps_sm = p("ps_sm", 2, "PSUM")


def build(dbg=False, sim=False):
    nc = bacc.Bacc("TRN2", target_bir_lowering=False, num_devices=NCORES)

    xT = nc.dram_tensor("xT", [F, N], BF16, kind="ExternalInput")
    seed = nc.dram_tensor("seed", [1, N], BF16, kind="ExternalInput")
    adjT = nc.dram_tensor("adjT", [N, N], BF16, kind="ExternalInput")
    adjT_osl = nc.dram_tensor("adjT_osl", [N, ISL], BF16, kind="ExternalInput")
    sel = nc.dram_tensor("sel", [N, ISL], BF16, kind="ExternalInput")
    w0_aug = nc.dram_tensor("w0_aug", [F, D + 1], BF16, kind="ExternalInput")
    w0a1 = nc.dram_tensor("w0a1", [F, 1], BF16, kind="ExternalInput")
    thw_aug = nc.dram_tensor("thw_aug", [1, D + 1], BF16, kind="ExternalInput")
    c01 = nc.dram_tensor("c01", [1, 1], BF16, kind="ExternalInput")
    w1_aug = nc.dram_tensor("w1_aug", [D, D + 1], BF16, kind="ExternalInput")
    w1a1 = nc.dram_tensor("w1a1", [D, 1], BF16, kind="ExternalInput")
    wo = nc.dram_tensor("wo", [D, OUT], BF16, kind="ExternalInput")
    vo = nc.dram_tensor("vo", [D, 2], BF16, kind="ExternalInput")
    identO = nc.dram_tensor("identO", [OUT, OUT], BF16, kind="ExternalInput")

    outT = nc.dram_tensor("outT", [OUT, ISL], FP32, kind="ExternalOutput")
    if dbg:
        h0_dbg = nc.dram_tensor("h0_dbg", [D, N], BF16, kind="ExternalOutput")
        h1_dbg = nc.dram_tensor("h1_dbg", [D, N], BF16, kind="ExternalOutput")
        who_dbg = nc.dram_tensor("who_dbg", [OUT, N], BF16,
                                 kind="ExternalOutput")
        fo_dbg = nc.dram_tensor("fo_dbg", [128, 2 * NJB], BF16,
                                kind="ExternalOutput")
        f1sl_dbg = nc.dram_tensor("f1sl_dbg", [1, ISL], BF16,
                                  kind="ExternalOutput")
        gb_dbg = nc.dram_tensor("gb_dbg", [128, ISL], BF16,
                                kind="ExternalOutput")
        agg_dbg = nc.dram_tensor("agg_dbg", [OUT + 1, ISL], FP32,
                                 kind="ExternalOutput")
        rinv_dbg = nc.dram_tensor("rinv_dbg", [1, ISL], FP32,
                                  kind="ExternalOutput")
        rbo_dbg = nc.dram_tensor("rbo_dbg", [OUT, ISL], FP32,
                                 kind="ExternalOutput")
        hpo_dbg = nc.dram_tensor("hpo_dbg", [OUT, ISL], FP32,
                                 kind="ExternalOutput")

    ar_in = [nc.dram_tensor("ar_in_a", [128, AW_A], BF16),
             nc.dram_tensor("ar_in_b", [128, AW_B], BF16)]
    ar_out = [nc.dram_tensor("ar_out_a", [128, AW_A], BF16,
                             addr_space="Shared"),
              nc.dram_tensor("ar_out_b", [128, AW_B], BF16,
                             addr_space="Shared")]
    f1sl_d = nc.dram_tensor("f1sl_d", [1, ISL], BF16)

    with tile.TileContext(nc) as tc, ExitStack() as ctx:
        b = Builder(nc, tc, ctx)
        st = b.state

        ones1 = st.tile([1, 128], BF16, tag="ones1", name="ones1")
        nc.vector.memset(ones1[:, :], 1.0)
        ones1f = st.tile([1, 128], FP32, tag="ones1f", name="ones1f")
        nc.vector.memset(ones1f[:, :], 1.0)
        ones128 = st.tile([128, 1], BF16, tag="ones128", name="ones128")
        nc.vector.memset(ones128[:, :], 1.0)

        def load_w(ap, shape, tag):
            s = st.tile(shape, BF16, tag=tag, name=tag)
            nc.sync.dma_start(s[:, :], ap)
            return s

        xT_sb = []
        for k in range(2):
            t = st.tile([128, N], BF16, tag=f"xT{k}", name="xT")
            nc.sync.dma_start(t[:, :], xT[k * 128:(k + 1) * 128, :])
            xT_sb.append(t)

        seed_sb = load_w(seed[:, :], [1, N], "seed")
        w0aug_sb = [load_w(w0_aug[k * 128:(k + 1) * 128, :], [128, D + 1],
                           f"w0aug{k}") for k in range(2)]
        w0a1_sb = [load_w(w0a1[k * 128:(k + 1) * 128, :], [128, 1],
                          f"w0a1_{k}") for k in range(2)]
        thw_sb = load_w(thw_aug[:, :], [1, D + 1], "thw")
        c01_sb = load_w(c01[:, :], [1, 1], "c01")
        w1aug_sb = load_w(w1_aug[:, :], [D, D + 1], "w1aug")
        w1a1_sb = load_w(w1a1[:, :], [D, 1], "w1a1")
        wo_sb = load_w(wo[:, :], [D, OUT], "wo")
        vo_sb = load_w(vo[:, :], [D, 2], "vo")
        identO_sb = load_w(identO[:, :], [OUT, OUT], "identO")

        # ---------- helpers ----------
        def project(hT_list, waug_list, whnm_t, f2col_t, jb_lo, jb_hi,
                    rank1=False):
            """whnm_t[:, jb*128:...] = (h @ W)_node-major; f2col_t[:, jb] = h@(W a2).
            rank1 adds seed_i * thw_aug (mergeState theta fold)."""
            nk = len(hT_list)
            W1c = D + 1
            for g0 in range(jb_lo, jb_hi, 3):
                ps = b.ps_sm.tile([128, 512], FP32, tag="sm", name="sm")
                for u_ in range(3):
                    jb = g0 + u_
                    sl = ps[:, u_ * W1c:(u_ + 1) * W1c]
                    for k in range(nk):
                        nc.tensor.matmul(
                            sl, hT_list[k][:, jb * 128:(jb + 1) * 128],
                            waug_list[k][:, :],
                            start=(k == 0),
                            stop=(k == nk - 1 and not rank1),
                        )
                    if rank1:
                        nc.tensor.matmul(
                            sl, seed_sb[:, jb * 128:(jb + 1) * 128],
                            thw_sb[:, :], start=False, stop=True,
                        )
                src_w = _ap(ps[:, :], 0, [ps.ap[0], [W1c, 3], [1, D]])
                dst_w = _ap(whnm_t[:, :], g0 * D,
                            [whnm_t.ap[0], [D, 3], [1, D]])
                nc.scalar.activation(dst_w, src_w, AF.Copy)
                src_f = _ap(ps[:, :], D, [ps.ap[0], [W1c, 3]])
                nc.scalar.activation(f2col_t[:, g0:g0 + 3], src_f, AF.Copy)

        def f_rows(hT_list, wa1_list, g_t, chunks, rank1=False):
            """Per 512-chunk: f1 row slice via PE, broadcast to 128
            partitions via a ones-outer-product matmul, then
            g = exp((alpha-1)*f1) straight out of PSUM on ACT.  The
            attention is computed up to the per-i factor P_i = exp(f1_i)
            (softmax normalization cancels it), so g is the only
            i-dependent tensor the elementwise chain needs."""
            nk = len(hT_list)
            for ch in chunks:
                ps1 = b.ps_rs.tile([1, 512], FP32, tag="rs2", name="rs2")
                for k in range(nk):
                    nc.tensor.matmul(
                        ps1[:, :], wa1_list[k][:, :],
                        hT_list[k][:, ch * 512:(ch + 1) * 512],
                        start=(k == 0), stop=(k == nk - 1 and not rank1),
                    )
                if rank1:
                    nc.tensor.matmul(
                        ps1[:, :], c01_sb[:, :],
                        seed_sb[:, ch * 512:(ch + 1) * 512],
                        start=False, stop=True,
                    )
                sl = slice(ch * 512, (ch + 1) * 512)
                f1c = b.misc.tile([1, 512], BF16, tag="f1row", name="f1c",
                                  bufs=2)
                nc.scalar.activation(f1c[:, :], ps1[:, :], AF.Copy)
                bcp = b.ps_sm.tile([128, 512], FP32, tag="sm", name="bcp")
                nc.tensor.matmul(bcp[:, :], ones1[:, :], f1c[:, :],
                                 start=True, stop=True)
                nc.scalar.activation(g_t[:, sl], bcp[:, :], AF.Exp,
                                     scale=ALPHA - 1.0)

        def exp_cols(f2col_t, pref):
            qc = st.tile([128, NJB], FP32, tag=pref + "q", name="qc")
            nc.scalar.activation(qc[:, :], f2col_t[:, :], AF.Exp, scale=ALPHA)
            Qc = st.tile([128, NJB], FP32, tag=pref + "Q", name="Qc")
            nc.scalar.activation(Qc[:, :], f2col_t[:, :], AF.Exp)
            return qc, Qc

        def attention(mask_dram, whnm_t, qc, Qc, g_bc,
                      h_out, between=None):
            for h in range(2):
                col0 = h * HALF
                agg_ps = [b.ps_agg.tile([D, 512], FP32, tag=f"agg{c}",
                                        name="agg") for c in range(3)]
                rs_ps = [b.ps_rs.tile([1, 512], FP32, tag=f"rs{c}",
                                      name="rs") for c in range(2)]
                rs_ps.append(b.ps_rs.tile([1, 512], FP32, tag="rs2",
                                          name="rs"))
                mt = None
                for jb in range(NJB):
                    if jb % MCH == 0:
                        mt = b.mask.tile([128, MCH * HALF], BF16, tag="mch",
                                         name="mch")
                        msrc = _ap(mask_dram[jb * 128:(jb + 1) * 128,
                                             col0:col0 + HALF], 0,
                                   [[N, 128], [128 * N, MCH], [1, HALF]])
                        mdst = _ap(mt[:, :], 0,
                                   [mt.ap[0], [HALF, MCH], [1, HALF]])
                        nc.sync.dma_start(mdst, msrc)
                    msl = mt[:, (jb % MCH) * HALF:(jb % MCH + 1) * HALF]
                    u = b.work.tile([128, HALF], BF16, tag="u", name="u")
                    ueng = nc.gpsimd if U_GP[jb] else nc.vector
                    ueng.tensor_scalar(
                        u[:, :], g_bc[:, col0:col0 + HALF],
                        qc[:, jb:jb + 1], Qc[:, jb:jb + 1],
                        ALU.mult, ALU.max)
                    at = b.att.tile([128, HALF], BF16, tag="at", name="at")
                    eng = nc.gpsimd if AT_GP[jb] else nc.vector
                    eng.tensor_tensor(at[:, :], u[:, :], msl, ALU.mult)
                    for c in range(3):
                        nc.tensor.matmul(
                            agg_ps[c][:, :], whnm_t[:, jb * D:(jb + 1) * D],
                            at[:, c * 512:(c + 1) * 512],
                            start=(jb == 0), stop=(jb == NJB - 1))
                    for c in range(3):
                        nc.tensor.matmul(
                            rs_ps[c][:, :], ones128[:, :],
                            at[:, c * 512:(c + 1) * 512],
                            start=(jb == 0), stop=(jb == NJB - 1))
                # epilogue: normalize + ELU -> h_out[:, col0:col0+HALF]
                rinvs = []
                for c in range(3):
                    rv = b.misc.tile([1, 512], FP32, tag="rinv",
                                     name="rinv", bufs=2)
                    nc.vector.reciprocal_approx_fast(rv[:, :], rs_ps[c][:, :])
                    rb = b.misc.tile([1, 512], BF16, tag="rinvb",
                                     name="rinvb", bufs=2)
                    nc.scalar.activation(rb[:, :], rv[:, :], AF.Copy)
                    rinvs.append(rb)
                hpn = b.misc.tile([D, HALF], BF16, tag="hpn", name="hpn")
                for c in range(3):
                    ps = b.ps_sm.tile([128, 512], FP32, tag="sm", name="sm")
                    nc.tensor.matmul(ps[:D, :], ones1[:, :D],
                                     rinvs[c][:, :],
                                     start=True, stop=True)
                    rbs = b.misc.tile([D, 512], FP32, tag="rbs", name="rbs",
                                      bufs=2)
                    nc.scalar.activation(rbs[:, :], ps[:D, :], AF.Copy)
                    nc.vector.tensor_tensor(
                        hpn[:, c * 512:(c + 1) * 512], agg_ps[c][:, :],
                        rbs[:, :], ALU.mult)
                mmin = b.misc.tile([D, HALF], BF16, tag="mmin", name="mmin")
                nc.vector.tensor_scalar(mmin[:, :], hpn[:, :], 0.0, None,
                                        ALU.min)
                ee = b.misc.tile([D, HALF], BF16, tag="ee", name="ee")
                nc.scalar.activation(ee[:, :], mmin[:, :], AF.Exp)
                rr = b.misc.tile([D, HALF], BF16, tag="rr", name="rr")
                nc.vector.tensor_scalar(rr[:, :], hpn[:, :], 0.0, -1.0,
                                        ALU.max, ALU.add)
                nc.vector.tensor_tensor(h_out[:, col0:col0 + HALF],
                                        ee[:, :], rr[:, :], ALU.add)
                if h == 0 and between is not None:
                    between()

        # ---------- layer 0 ----------
        whnm0 = st.tile([128, NJB * D], BF16, tag="whnm", name="whnm0",
                        bufs=2)
        f2c0 = st.tile([128, NJB], FP32, tag="f2col", name="f2c0", bufs=2)
        gbc0 = b.bc.tile([128, N], BF16, tag="g_bc", name="gbc0")
        f_rows(xT_sb, w0a1_sb, gbc0, range(6), rank1=True)
        project(xT_sb, w0aug_sb, whnm0, f2c0, 0, NJB, rank1=True)
        qc0, Qc0 = exp_cols(f2c0, "l0")
        h0T = st.tile([D, N], BF16, tag="h0T", name="h0T")

        whnm1 = st.tile([128, NJB * D], BF16, tag="whnm", name="whnm1",
                        bufs=2)
        f2c1 = st.tile([128, NJB], FP32, tag="f2col", name="f2c1", bufs=2)
        gbc1 = b.bc.tile([128, N], BF16, tag="g_bc", name="gbc1")

        def emit_l1_early():
            # h0T cols < HALF fully emitted after half 0: project jb 0..11
            # and build the first half of layer-1's f1/P/p rows
            project([h0T], [w1aug_sb], whnm1, f2c1, 0, NJB // 2)
            f_rows([h0T], [w1a1_sb], gbc1, range(3))

        attention(adjT, whnm0, qc0, Qc0, gbc0, h0T,
                  between=emit_l1_early)

        # ---------- layer 1 ----------
        project([h0T], [w1aug_sb], whnm1, f2c1, NJB // 2, NJB)
        qc1, Qc1 = exp_cols(f2c1, "l1")
        f_rows([h0T], [w1a1_sb], gbc1, range(3, 6))
        h1T = st.tile([D, N], BF16, tag="h1T", name="h1T")

        fo_sb = st.tile([128, 2 * NJB], BF16, tag="fo", name="fo_sb")
        # sel shares its tag with a tiny gate tile whose late memset pins the
        # big selector DMA into the layer-1 window (WAW on the shared buffer)
        sel_gate = st.tile([1, 8], BF16, tag="sel", name="sel_gate")

        def who_fo_stage(h):
            """who chunk + fo projections for half h, stage into ar_in."""
            col0 = h * HALF
            who_sb = b.misc.tile([OUT, HALF], BF16, tag="who", name="who_sb",
                                 bufs=2)
            for c in range(3):
                ps = b.ps_sm.tile([128, 512], FP32, tag="sm", name="sm")
                nc.tensor.matmul(ps[:OUT, :], wo_sb[:, :],
                                 h1T[:, col0 + c * 512:col0 + (c + 1) * 512],
                                 start=True, stop=True)
                nc.scalar.activation(who_sb[:, c * 512:(c + 1) * 512],
                                     ps[:OUT, :], AF.Copy)
            ps = b.ps_sm.tile([128, 512], FP32, tag="sm", name="sm")
            for u_ in range(12):
                jb = h * 12 + u_
                nc.tensor.matmul(ps[:, u_ * 2:(u_ + 1) * 2],
                                 h1T[:, jb * 128:(jb + 1) * 128], vo_sb[:, :],
                                 start=True, stop=True)
            nc.scalar.activation(fo_sb[:, h * NJB:(h + 1) * NJB],
                                 ps[:, :NJB], AF.Copy)
            # stage: fo slice + who half
            arx, aw = ar_in[h], (AW_A, AW_B)[h]
            nc.sync.dma_start(
                _ap(arx[:1, :], FO_OFF, [[aw, 128], [1, NJB]]),
                fo_sb[:, h * NJB:(h + 1) * NJB])
            nc.sync.dma_start(
                _ap(arx[:1, :], WHO_OFF,
                    [[2 * aw, OUT], [aw, 2], [1, 768]]),
                _ap(who_sb[:, :], 0, [who_sb.ap[0], [768, 2], [1, 768]]))

        groups = [list(range(NCORES))]

        def emit_ar(h):
            if sim:
                nc.sync.dma_start(ar_out[h][:, :], ar_in[h][:, :])
            else:
                nc.gpsimd.collective_compute(
                    "AllReduce", ALU.add, replica_groups=groups,
                    ins=[ar_in[h].ap().opt()], outs=[ar_out[h].ap().opt()],
                )

        sel_sb = None

        def emit_who_early():
            nonlocal sel_sb
            who_fo_stage(0)
            emit_ar(0)
            # selector load, WAW-pinned behind the gate memset so the
            # scheduler cannot hoist its transfer into the startup window
            nc.gpsimd.memset(sel_gate[:, :], 0.0)
            sel_sb = st.tile([128, NJB * ISL], BF16, tag="sel",
                             name="sel_sb")
            nc.sync.dma_start(
                _ap(sel_sb[:, :], 0, [sel_sb.ap[0], [ISL, NJB], [1, ISL]]),
                _ap(sel[:128, :], 0, [[ISL, 128], [128 * ISL, NJB],
                                      [1, ISL]]))

        attention(adjT, whnm1, qc1, Qc1, gbc1, h1T,
                  between=emit_who_early)

        who_fo_stage(1)
        emit_ar(1)
        if dbg:
            nc.sync.dma_start(h0_dbg[:, :], h0T[:, :])
            nc.sync.dma_start(h1_dbg[:, :], h1T[:, :])
        # out-layer mask: borrows a mask-pool buffer (its WAR dependency on
        # late layer-1 chunks lands the transfer at the end of layer 1)
        osl_sb = b.mask.tile([128, NJB * ISL], BF16, tag="mch",
                             name="osl_sb")
        nc.sync.dma_start(
            _ap(osl_sb[:, :], 0, [osl_sb.ap[0], [ISL, NJB], [1, ISL]]),
            _ap(adjT_osl[:128, :], 0, [[ISL, 128], [128 * ISL, NJB],
                                       [1, ISL]]))

        # ---------- out layer ----------
        whTo = st.tile([OUT, N], BF16, tag="whTo", name="whTo")
        fo_out = st.tile([128, 2 * NJB], BF16, tag="fo_out", name="fo_out")
        whnmo = st.tile([128, NJB * OUT], BF16, tag="whnmo", name="whnmo")
        qco = st.tile([128, NJB], FP32, tag="qco", name="qco")
        Qco = st.tile([128, NJB], FP32, tag="Qco", name="Qco")
        sel_ps = b.ps_rs.tile([1, 512], FP32, tag="rs0", name="sel_ps")

        def out_half(h):
            lo_jb, hi_jb = (0, 12) if h == 0 else (12, NJB)
            arx, aw = ar_out[h], (AW_A, AW_B)[h]
            nc.sync.dma_start(
                _ap(whTo[:, :], h * HALF, [whTo.ap[0], [768, 2], [1, 768]]),
                _ap(arx[:1, :], WHO_OFF, [[2 * aw, OUT], [aw, 2], [1, 768]]))
            nc.sync.dma_start(
                fo_out[:, h * NJB:(h + 1) * NJB],
                _ap(arx[:1, :], FO_OFF, [[aw, 128], [1, NJB]]))
            for jb in range(lo_jb, hi_jb):
                ps = b.ps_sm.tile([128, OUT], BF16, tag="sm", name="smT")
                nc.tensor.transpose(ps[:, :],
                                    whTo[:, jb * 128:(jb + 1) * 128],
                                    identO_sb[:, :])
                nc.scalar.activation(
                    whnmo[:, jb * OUT:(jb + 1) * OUT], ps[:, :], AF.Copy)
            # f2-derived cols for this half's jb range (strided src: odd cols)
            w = hi_jb - lo_jb
            src = _ap(fo_out[:, :], 2 * lo_jb + 1, [fo_out.ap[0], [2, w]])
            nc.scalar.activation(qco[:, lo_jb:hi_jb], src, AF.Exp,
                                 scale=ALPHA)
            nc.scalar.activation(Qco[:, lo_jb:hi_jb], src, AF.Exp)
            # f1o select for the own i-slice (sum over cores happened in the
            # AllReduce; the one-hot select must come after it)
            for jb in range(lo_jb, hi_jb):
                nc.tensor.matmul(
                    sel_ps[:, :ISL], fo_out[:, 2 * jb:2 * jb + 1],
                    sel_sb[:, jb * ISL:(jb + 1) * ISL],
                    start=(jb == 0), stop=(jb == NJB - 1))

        out_half(0)
        out_half(1)
        if dbg:
            nc.sync.dma_start(who_dbg[:, :], whTo[:, :])

        # f1o own-slice -> row -> broadcast
        f1sl = b.misc.tile([1, ISL], BF16, tag="f1sl", name="f1sl")
        nc.scalar.activation(f1sl[:, :], sel_ps[:, :ISL], AF.Copy)
        if dbg:
            nc.sync.dma_start(fo_dbg[:, :], fo_out[:, :])
            nc.sync.dma_start(f1sl_dbg[:, :], f1sl[:, :])
        ps = b.ps_sm.tile([128, 512], FP32, tag="sm", name="sm")
        nc.tensor.matmul(ps[:, :ISL], ones1[:, :], f1sl[:, :],
                         start=True, stop=True)
        gbco = b.misc.tile([128, ISL], BF16, tag="gbco", name="gbco")
        nc.scalar.activation(gbco[:, :], ps[:, :ISL], AF.Exp,
                             scale=ALPHA - 1.0)
        if dbg:
            nc.sync.dma_start(gb_dbg[:, :], gbco[:, :])

        agg_o = b.ps_agg.tile([OUT, ISL], FP32, tag="agg0", name="agg_o")
        rs_o = b.ps_agg.tile([1, ISL], FP32, tag="agg1", name="rs_o")
        for jb in range(NJB):
            msl = osl_sb[:, jb * ISL:(jb + 1) * ISL]
            u = b.work.tile([128, ISL], BF16, tag="u", name="uo")
            ueng = nc.gpsimd if U_GP[jb] else nc.vector
            ueng.tensor_scalar(u[:, :], gbco[:, :], qco[:, jb:jb + 1],
                               Qco[:, jb:jb + 1], ALU.mult, ALU.max)
            at = b.att.tile([128, ISL], BF16, tag="at", name="ato")
            eng = nc.gpsimd if AT_GP[jb] else nc.vector
            eng.tensor_tensor(at[:, :], u[:, :], msl, ALU.mult)
            nc.tensor.matmul(
                agg_o[:, :], whnmo[:, jb * OUT:(jb + 1) * OUT],
                at[:, :], start=(jb == 0), stop=(jb == NJB - 1))
            nc.tensor.matmul(
                rs_o[:, :], ones128[:, :],
                at[:, :], start=(jb == 0), stop=(jb == NJB - 1))

        if dbg:
            aggc = b.misc.tile([OUT, ISL], FP32, tag="aggc", name="aggc")
            nc.scalar.activation(aggc[:, :], agg_o[:, :], AF.Copy)
            nc.sync.dma_start(agg_dbg[:OUT, :], aggc[:, :])
            denc = b.misc.tile([1, ISL], FP32, tag="denc", name="denc")
            nc.scalar.activation(denc[:, :], rs_o[:, :], AF.Copy)
            nc.sync.dma_start(agg_dbg[OUT:, :], denc[:, :])
        rinv_o = b.misc.tile([1, ISL], FP32, tag="rinvo", name="rinv_o")
        nc.vector.reciprocal_approx_fast(rinv_o[:, :], rs_o[:, :])
        ps = b.ps_sm.tile([128, 512], FP32, tag="sm", name="sm")
        nc.tensor.matmul(ps[:OUT, :ISL], ones1f[:, :OUT], rinv_o[:, :],
                         start=True, stop=True)
        rbo = b.misc.tile([OUT, ISL], FP32, tag="rbo", name="rbo")
        nc.scalar.activation(rbo[:, :], ps[:OUT, :ISL], AF.Copy)
        hpo = b.misc.tile([OUT, ISL], FP32, tag="hpo", name="hpo")
        nc.vector.tensor_tensor(hpo[:, :], agg_o[:OUT, :], rbo[:, :],
                                ALU.mult)
        if dbg:
            nc.sync.dma_start(rinv_dbg[:, :], rinv_o[:, :])
            nc.sync.dma_start(rbo_dbg[:, :], rbo[:, :])
            nc.sync.dma_start(hpo_dbg[:, :], hpo[:, :])
        mo = b.misc.tile([OUT, ISL], FP32, tag="mo", name="mo")
        nc.vector.tensor_scalar(mo[:, :], hpo[:, :], 0.0, None, ALU.min)
        eo = b.misc.tile([OUT, ISL], FP32, tag="eo", name="eo")
        nc.scalar.activation(eo[:, :], mo[:, :], AF.Exp)
        ro = b.misc.tile([OUT, ISL], FP32, tag="ro", name="ro")
        nc.vector.tensor_scalar(ro[:, :], hpo[:, :], 0.0, -1.0, ALU.max,
                                ALU.add)
        fin = b.misc.tile([OUT, ISL], FP32, tag="fin", name="fin")
        nc.vector.tensor_add(fin[:, :], eo[:, :], ro[:, :])
        nc.sync.dma_start(outT[:, :], fin[:, :])
    nc.compile()
    return nc


def make_in_maps(inputs):
    x = np.asarray(inputs["x"], np.float32)
    adj = np.asarray(inputs["adj"], np.float32)
    observation = np.asarray(inputs["observation"])
    theta = np.asarray(inputs["theta"], np.float32)
    W0 = np.asarray(inputs["W0"], np.float32)
    a0 = np.asarray(inputs["a0"], np.float32)
    W1 = np.asarray(inputs["W1"], np.float32)
    a1 = np.asarray(inputs["a1"], np.float32)
    Wo = np.asarray(inputs["Wo"], np.float32)
    ao = np.asarray(inputs["ao"], np.float32)

    bf = ml_dtypes.bfloat16
    xT = np.ascontiguousarray(x.T).astype(bf)
    seed = (observation[0] == 1).astype(np.float32)[None, :].astype(bf)
    adjT = np.ascontiguousarray((adj > 0).T.astype(np.float32)).astype(bf)
    identO = np.eye(OUT, dtype=np.float32).astype(bf)

    in_maps = []
    for c in range(NCORES):
        w0a2 = W0[c] @ a0[c][D:]           # [F, 1]
        w0a1_ = W0[c] @ a0[c][:D]
        thw = theta @ W0[c]                # [1, D]
        c02 = float((theta @ w0a2).item())
        c01_ = float((theta @ w0a1_).item())
        w1a2 = W1[c] @ a1[c][D:]
        w1a1_ = W1[c] @ a1[c][:D]
        wo_c = Wo[c * D:(c + 1) * D]       # [D, OUT]
        vo_c = np.concatenate([wo_c @ ao[:OUT], wo_c @ ao[OUT:]], axis=1)
        sel_c = np.zeros((N, ISL), np.float32)
        sel_c[c * ISL + np.arange(ISL), np.arange(ISL)] = 1.0
        in_maps.append({
            "xT": xT, "seed": seed, "adjT": adjT,
            "adjT_osl": np.ascontiguousarray(adjT[:, c * ISL:(c + 1) * ISL]),
            "sel": sel_c.astype(bf),
            "w0_aug": np.concatenate([W0[c], w0a2], axis=1).astype(bf),
            "w0a1": w0a1_.astype(bf),
            "thw_aug": np.concatenate([thw, [[c02]]], axis=1).astype(bf),
            "c01": np.array([[c01_]], np.float32).astype(bf),
            "w1_aug": np.concatenate([W1[c], w1a2], axis=1).astype(bf),
            "w1a1": w1a1_.astype(bf),
            "wo": wo_c.astype(bf),
            "vo": vo_c.astype(bf),
            "identO": identO,
        })
    return in_maps


def kernel(**inputs):
    in_maps = make_in_maps(inputs)
    nc = build()
    res = run_bass_kernel_spmd(nc, in_maps, core_ids=list(range(NCORES)))
    out = np.concatenate(
        [res.results[c]["outT"].T for c in range(NCORES)], axis=0
    )
    return np.ascontiguousarray(out, np.float32)


if __name__ == "__main__":
    build()
    print("built ok")
